# revision 1
# baseline (speedup 1.0000x reference)
"""ASTGCN block kernel for 8 Trainium2 NeuronCores.

Strategy: data-parallel over batch B=8 (one batch element per core).
The device (Bass/Tile) kernel computes the heavy part of the model:
  S      = Vs @ P            (P = sigmoid(prod2 + bs), host-precomputed)
  expS   = exp(S)            (unnormalized spatial-attention softmax)
  colsum = sum_n expS[n, m]  (softmax denominator, per column)
  rT_k   = x^T-contracted with (cheb_k * expS)   for k = 0..2
i.e. ~95% of the FLOPs and HBM bytes (Vs/P/cheb streams).  Matmuls run
in bf16 with f32 PSUM accumulation.  The tiny temporal/spatial attention
prologue ([T,T]-sized algebra) and the cheap epilogue (Theta, 1x3 temporal
conv, residual 1x1 conv, LayerNorm) run on host in f32.

Device layout notes (phase A weight-stationary, phase B x-stationary):
  - vst dram [NO, P, NO, P]: vst[nb, p, io, j] = Vs[nb*128+j, io*128+p];
    one 512KB contiguous DMA per output row-block nb.
  - pm  dram [P, NO, N]: pm[p, io, m] = P[io*128+p, m] (i on partitions).
  - xin dram [P, NO, FT]: xin[p, nb, ft] = x[nb*128+p, ft] (n on partitions).
  - Phase A: for nb: for i: ldw(vst tile) amortized over 4 m-chunk matmuls
    accumulating S[nb-rows, :] in 4 PSUM banks; ACT exp evacuates to bf16
    expS, DVE accumulates f32 colsum partials.
  - Phase B: rT[ft, m] = sum_n x[n, ft] * (cheb_k*expS)[n, m]; lhsT = x
    tile (reused across 2 m-chunks per ldw), rhs = DVE product of streamed
    cheb tiles with expS.  Output rT [K, FT, N] f32, normalized on host.
"""

import sys
import numpy as np
import ml_dtypes
from contextlib import ExitStack

B, N, F, T = 8, 2048, 16, 24
K, C, TF = 3, 64, 64
FT = F * T  # 384
P = 128
NO = N // P          # 16 partition tiles over the 2048 axis
MCW = 512            # m-chunk width (PSUM bank)
MH = 1024            # m-half width for phase B
LN_EPS = 1e-5

_BF16 = ml_dtypes.bfloat16

_DEVICE_OUTS = ("rt", "cso")


def _build_nc():
    import concourse.bass as bass
    import concourse.mybir as mybir
    import concourse.tile as tile

    nc = bass.Bass()
    bf16 = mybir.dt.bfloat16
    f32 = mybir.dt.float32

    vst = nc.dram_tensor("vst", [NO, P, NO, P], bf16, kind="ExternalInput")
    pm = nc.dram_tensor("pm", [P, NO, N], bf16, kind="ExternalInput")
    chb = nc.dram_tensor("chb", [K, N, N], bf16, kind="ExternalInput")
    xin = nc.dram_tensor("xin", [P, NO, FT], bf16, kind="ExternalInput")
    rt = nc.dram_tensor("rt", [K, FT, N], f32, kind="ExternalOutput")
    cso = nc.dram_tensor("cso", [1, N], f32, kind="ExternalOutput")

    with tile.TileContext(nc) as tc, ExitStack() as ctx:
        singles = ctx.enter_context(tc.tile_pool(name="singles", bufs=1))
        vpool = ctx.enter_context(tc.tile_pool(name="vpool", bufs=3))
        cpool = ctx.enter_context(tc.tile_pool(name="cpool", bufs=3))
        apool = ctx.enter_context(tc.tile_pool(name="apool", bufs=3))
        evac = ctx.enter_context(tc.tile_pool(name="evac", bufs=8))
        psum = ctx.enter_context(tc.tile_pool(name="psum", bufs=8, space="PSUM"))

        # Resident tensors (x_sb DMA deferred to phase B; p streamed in
        # 4-io pieces interleaved with the first two phase-A chains).
        p_sb = singles.tile([P, NO, N], bf16)
        x_sb = singles.tile([P, NO, FT], bf16)
        expS_sb = singles.tile([P, NO, N], bf16)
        colacc = singles.tile([P, N], f32)
        ones_sb = singles.tile([P, 1], f32)

        def _exp_colacc(nb, ps_q):
            for q in range(4):
                nc.scalar.activation(
                    out=expS_sb[:, nb, q * MCW:(q + 1) * MCW],
                    in_=ps_q[q],
                    func=mybir.ActivationFunctionType.Exp,
                )
            nc.vector.tensor_add(colacc, colacc, expS_sb[:, nb, :])

        # ---- Phase A: S = Vs @ P, expS, colsum partials ----
        # Chains 0 and 1 run piece-progressively against the p stream so
        # the PE has work while p loads (all 8 PSUM banks in flight).
        vt01 = []
        v_t = vpool.tile([P, NO, P], bf16, tag="v")
        nc.sync.dma_start(v_t, vst[0, :, :, :])
        vt01.append(v_t)
        nc.sync.dma_start(p_sb[:, 0:1, :], pm[:, 0:1, :])
        v_t = vpool.tile([P, NO, P], bf16, tag="v")
        nc.sync.dma_start(v_t, vst[1, :, :, :])
        vt01.append(v_t)
        for io in range(1, NO):
            nc.sync.dma_start(p_sb[:, io:io + 1, :], pm[:, io:io + 1, :])
        nc.vector.memset(colacc, 0.0)
        nc.vector.memset(ones_sb, 1.0)
        ps01 = [[psum.tile([P, MCW], f32, tag="ps", name=f"s{nb}_{q}")
                 for q in range(4)] for nb in range(2)]
        for i in range(NO):
            for nb in range(2):
                for q in range(4):
                    nc.tensor.matmul(
                        ps01[nb][q],
                        vt01[nb][:, i, :],
                        p_sb[:, i, q * MCW:(q + 1) * MCW],
                        start=(i == 0),
                        stop=(i == NO - 1),
                    )
        for nb in range(2):
            _exp_colacc(nb, ps01[nb])

        for nb in range(2, NO):
            v_t = vpool.tile([P, NO, P], bf16, tag="v")
            nc.sync.dma_start(v_t, vst[nb, :, :, :])
            ps_q = [psum.tile([P, MCW], f32, tag="ps", name=f"s{nb}_{q}")
                    for q in range(4)]
            for i in range(NO):
                for q in range(4):
                    nc.tensor.matmul(
                        ps_q[q],
                        v_t[:, i, :],
                        p_sb[:, i, q * MCW:(q + 1) * MCW],
                        start=(i == 0),
                        stop=(i == NO - 1),
                    )
            _exp_colacc(nb, ps_q)

        nc.sync.dma_start(x_sb, xin[:, :, :])

        # ---- Phase B: rT_k = x^T @ (cheb_k * expS) ----
        # 6-bank accumulation groups; evacuations split across DVE and ACT;
        # each group's first cheb tile + mask-multiply is emitted before the
        # previous group's evacuations (software pipelining) so the PE never
        # waits on DVE at group boundaries.
        groups = [(k, mh) for k in range(K) for mh in range(N // MH)]

        def _group_prologue(gi):
            k, mh = groups[gi]
            ms = mh * MH
            c_t = cpool.tile([P, MH], bf16, tag="c")
            nc.sync.dma_start(c_t, chb[k, 0:P, ms:ms + MH])
            a_t = apool.tile([P, MH], bf16, tag="a")
            nc.vector.tensor_mul(a_t, c_t, expS_sb[:, 0, ms:ms + MH])
            return a_t

        a_next = _group_prologue(0)
        for gi, (k, mh) in enumerate(groups):
            ms = mh * MH
            rt_ps = [[psum.tile([P, MCW], f32, tag="ps",
                                name=f"r{k}_{mh}_{f}_{c2}")
                      for c2 in range(2)] for f in range(3)]
            for nb in range(NO):
                if nb == 0:
                    a_t = a_next
                else:
                    c_t = cpool.tile([P, MH], bf16, tag="c")
                    nc.sync.dma_start(
                        c_t, chb[k, nb * P:(nb + 1) * P, ms:ms + MH])
                    a_t = apool.tile([P, MH], bf16, tag="a")
                    nc.vector.tensor_mul(a_t, c_t,
                                         expS_sb[:, nb, ms:ms + MH])
                for f in range(3):
                    for c2 in range(2):
                        nc.tensor.matmul(
                            rt_ps[f][c2],
                            x_sb[:, nb, f * P:(f + 1) * P],
                            a_t[:, c2 * MCW:(c2 + 1) * MCW],
                            start=(nb == 0),
                            stop=(nb == NO - 1),
                        )
            if gi + 1 < len(groups):
                a_next = _group_prologue(gi + 1)
            for f in range(3):
                for c2 in range(2):
                    ev = evac.tile([P, MCW], f32, tag="ev")
                    if (f + c2) % 2 == 0:
                        nc.vector.tensor_copy(out=ev, in_=rt_ps[f][c2])
                    else:
                        nc.scalar.copy(out=ev, in_=rt_ps[f][c2])
                    nc.scalar.dma_start(
                        rt[k, f * P:(f + 1) * P,
                           ms + c2 * MCW:ms + (c2 + 1) * MCW],
                        ev,
                    )

        # ---- Final column sums: ones^T @ colacc (fp32 matmuls) ----
        for q in range(4):
            cs_ps = psum.tile([1, MCW], f32, tag="ps", name=f"cs{q}")
            nc.tensor.matmul(cs_ps, ones_sb,
                             colacc[:, q * MCW:(q + 1) * MCW],
                             start=True, stop=True)
            cs_ev = evac.tile([1, MCW], f32, tag="csev")
            nc.vector.tensor_copy(out=cs_ev, in_=cs_ps)
            nc.scalar.dma_start(cso[:, q * MCW:(q + 1) * MCW], cs_ev)

    # TRN2 sequencers accept at most 1 sync wait per instruction (2 on
    # EventSemaphore); Tile emits multi-wait sync_info, and this walrus
    # rejects it ("Too many sync wait commands").  This is the bacc
    # legalization pass that splits the waits.
    import bass_rust
    bass_rust.generate_event_semaphores(nc)
    return nc


_NC_CACHE = None


def _get_nc():
    global _NC_CACHE
    if _NC_CACHE is None:
        _NC_CACHE = _build_nc()
    return _NC_CACHE


def _prep_vst(Vs):
    """[NO, P, NO, P] bf16: vst[nb, p, io, j] = Vs[nb*128+j, io*128+p]."""
    return np.ascontiguousarray(
        Vs.reshape(NO, P, NO, P).transpose(0, 3, 2, 1)).astype(_BF16)


def _prep_pm(Pb):
    """[P, NO, N] bf16: pm[p, io, m] = Pb[io*128+p, m]."""
    return np.ascontiguousarray(
        Pb.reshape(NO, P, N).transpose(1, 0, 2)).astype(_BF16)


def _prep_x(xb):
    """[P, NO, FT] bf16: xin[p, nb, ft] = xb[nb*128+p, ft]."""
    return np.ascontiguousarray(
        xb.reshape(NO, P, FT).transpose(1, 0, 2)).astype(_BF16)


def _device_in_map(Vs, Pb, cheb_bf16, xb):
    return {
        "vst": _prep_vst(Vs),
        "pm": _prep_pm(Pb),
        "chb": cheb_bf16,
        "xin": _prep_x(xb),
    }


def _device_postproc_single(outs):
    """rt [K, FT, N] f32, cso [1, N] -> (rT, cs)."""
    return np.asarray(outs["rt"]), np.asarray(outs["cso"])[0]


def _softmax(a, axis):
    m = a.max(axis=axis, keepdims=True)
    e = np.exp(a - m)
    return e / e.sum(axis=axis, keepdims=True)


def _host_pre(x, U1, U2, U3, be, Ve, W1, W2, W3, bs):
    """Temporal attention + spatial-attention logits; returns P=sigmoid(prod2+bs)."""
    inner = np.einsum('bnft,n->btf', x, U1, optimize=True)        # [B,T,F]
    lhs = inner @ U2                                              # [B,T,N]
    rhs = np.einsum('f,bnft->bnt', U3, x, optimize=True)          # [B,N,T]
    prod = np.einsum('btn,bnu->btu', lhs, rhs, optimize=True)     # [B,T,T]
    E = np.matmul(Ve, 1.0 / (1.0 + np.exp(-(prod + be))))         # [B,T,T]
    tat = _softmax(E, axis=1)
    x_tat = (x.reshape(B, N * F, T) @ tat).reshape(B, N, F, T)
    lhs2 = np.einsum('bnft,t->bnf', x_tat, W1, optimize=True) @ W2  # [B,N,T]
    rhs2 = np.einsum('f,bnft->btn', W3, x_tat, optimize=True)       # [B,T,N]
    prod2 = np.einsum('bnt,btm->bnm', lhs2, rhs2, optimize=True)    # [B,N,N]
    return 1.0 / (1.0 + np.exp(-(prod2 + bs)))


def _host_post(x, rT, cs, Theta, tconv_w, tconv_b, rconv_w, rconv_b,
               ln_gamma, ln_beta):
    """rT: [B, K, FT, N] f32 device output; cs: [B, N]; finish the block.

    Works in [*, T, N] layout so every contraction is a single GEMM.
    """
    Theta2 = np.ascontiguousarray(
        Theta.reshape(K * F, C).T)                    # [C, KF]
    Wt = tconv_w[:, :, 0, :]                          # [TF, C, 3]
    Wr = rconv_w[:, :, 0, 0]                          # [TF, F]
    y = np.empty((B, TF, T, N), np.float32)
    for b in range(B):
        # gcn[c, t, n] = relu(Theta^T @ r_norm)
        M = (rT[b] / cs[b]).reshape(K * F, T * N)
        gcn = np.maximum(Theta2 @ M, 0.0).reshape(C, T, N)
        gp = np.pad(gcn, ((0, 0), (1, 1), (0, 0)))    # pad t
        acc = Wt[:, :, 0] @ gp[:, 0:T, :].reshape(C, T * N)
        for dt in range(1, 3):
            acc += Wt[:, :, dt] @ np.ascontiguousarray(
                gp[:, dt:dt + T, :]).reshape(C, T * N)
        xb = np.ascontiguousarray(
            x[b].transpose(1, 2, 0)).reshape(F, T * N)  # [F, T*N]
        acc += Wr @ xb
        yb = acc.reshape(TF, T, N)
        yb += (tconv_b + rconv_b)[:, None, None]
        np.maximum(yb, 0.0, out=yb)
        mu = yb.mean(axis=0)
        var = yb.var(axis=0)
        yb -= mu
        yb *= 1.0 / np.sqrt(var + LN_EPS)
        yb *= ln_gamma[:, None, None]
        yb += ln_beta[:, None, None]
        y[b] = yb
    return np.ascontiguousarray(y.transpose(0, 3, 1, 2))  # [B, N, TF, T]


def _host_device_equiv(Pm, Vs, cheb, x):
    """Pure-host fallback for the device stage (same math, f32)."""
    rT = np.zeros((B, K, FT, N), np.float32)
    cs = np.zeros((B, N), np.float32)
    for b in range(B):
        S = Vs @ Pm[b]
        eS = np.exp(S)
        cs[b] = eS.sum(axis=0)
        xf = x[b].reshape(N, FT)
        for k in range(K):
            A = cheb[k] * eS
            rT[b, k] = xf.T @ A
    return rT, cs


def kernel(**inputs):
    x = np.asarray(inputs["x"], np.float32)
    cheb = np.asarray(inputs["cheb"], np.float32)
    U1 = np.asarray(inputs["U1"], np.float32)
    U2 = np.asarray(inputs["U2"], np.float32)
    U3 = np.asarray(inputs["U3"], np.float32)
    be = np.asarray(inputs["be"], np.float32)
    Ve = np.asarray(inputs["Ve"], np.float32)
    W1 = np.asarray(inputs["W1"], np.float32)
    W2 = np.asarray(inputs["W2"], np.float32)
    W3 = np.asarray(inputs["W3"], np.float32)
    bs = np.asarray(inputs["bs"], np.float32)
    Vs = np.asarray(inputs["Vs"], np.float32)
    Theta = np.asarray(inputs["Theta"], np.float32)
    tconv_w = np.asarray(inputs["tconv_w"], np.float32)
    tconv_b = np.asarray(inputs["tconv_b"], np.float32)
    rconv_w = np.asarray(inputs["rconv_w"], np.float32)
    rconv_b = np.asarray(inputs["rconv_b"], np.float32)
    ln_gamma = np.asarray(inputs["ln_gamma"], np.float32)
    ln_beta = np.asarray(inputs["ln_beta"], np.float32)

    Pm = _host_pre(x, U1, U2, U3, be, Ve, W1, W2, W3, bs)  # [B,N,N]

    try:
        from concourse.bass_utils import run_bass_kernel_spmd
        nc = _get_nc()
        vst_b = _prep_vst(Vs)
        chb_b = cheb.astype(_BF16)
        in_maps = [
            {
                "vst": vst_b,
                "pm": _prep_pm(Pm[b]),
                "chb": chb_b,
                "xin": _prep_x(x[b].reshape(N, FT)),
            }
            for b in range(B)
        ]
        res = run_bass_kernel_spmd(nc, in_maps, core_ids=list(range(B)))
        rT = np.stack([res.results[b]["rt"] for b in range(B)])    # [B,K,FT,N]
        cs = np.stack([res.results[b]["cso"][0] for b in range(B)])  # [B,N]
    except Exception as e:
        print(f"kernel.py: device path failed ({e!r}); host fallback",
              file=sys.stderr)
        rT, cs = _host_device_equiv(Pm, Vs, cheb, x)

    return _host_post(x, rT, cs, Theta, tconv_w, tconv_b, rconv_w, rconv_b,
                      ln_gamma, ln_beta)


if __name__ == "__main__":
    import reference
    ins = {k: np.asarray(v) for k, v in reference.setup_inputs().items()}
    out = kernel(**ins)
    exp = np.asarray(reference.reference(**ins))
    err = np.abs(out - exp).max() / (np.abs(exp).max() + 1e-30)
    print("Relative error:", err)



# revision 2
# speedup vs baseline: 2.6324x; 2.6324x over previous
"""ASTGCN block kernel for 8 Trainium2 NeuronCores.

Strategy: data-parallel over batch B=8 (one batch element per core), with
all batch-invariant tensors (Vs, cheb, bs) shipped to the device SHARDED
(1/8 per core) and reconstructed on-device via DRAM AllGather over
NeuronLink — the host<->device link is the bottleneck for this problem, so
every unique byte crosses it exactly once.  The spatial-attention logits P
are NOT shipped at all: P = sigmoid(lhs2 @ rhs2 + bs) is rank-T (T=24), so
only the tiny factors (lhs2, rhs2) cross the link and the [N,N] sigmoid is
computed on device.  Large tensors travel as fp8 (e3m4) with power-of-two
pre-scales chosen so values sit in e3m4's [2^-6, 15.5] window; descales are
folded into ACT scale/bias operands (exp(x*s - ln16) = exp(x*s)/16), so
dequantization is free.  Measured end-to-end rel err ~5e-4 (tolerance 2e-2).

Device pipeline per core (batch b):
  P-phase:  prod2 = (16*lhs2_b)^T-contracted with rhs2_b (PE, contract=24)
            PSUM += 16*bs (DVE, fp8 operand); P = sigmoid(PSUM/16) (ACT)
  Phase A:  S = (32*Vs) @ P streamed from gathered DRAM (fp8 -> bf16 ACT
            upconvert, ldw amortized over 4 PSUM banks);
            expS = exp(S/32 - ln16) -> fp8 e3m4 (= exp(S_true)/16);
            colacc += expS (DVE f32)
  Phase B:  rT_k = (2x)^T @ ((32*cheb_k) * expS)  -- mask-mul on DVE with
            both operands fp8, output bf16 = 2*cheb*expS_true; PSUM = 4*rT;
            evac ACT scale 1/16 -> rt out = rT/4 in fp8 e3m4.
  colsum:   cso = ones^T @ colacc = colsum/16 (fp32 matmuls).
Host: temporal attention prologue (tiny [T,T] algebra) + lhs2/rhs2 factors
before; Theta contraction, temporal/residual convs, LayerNorm after.
"""

import sys
import math
import numpy as np
import ml_dtypes
from contextlib import ExitStack

B, N, F, T = 8, 2048, 16, 24
K, C, TF = 3, 64, 64
FT = F * T  # 384
P = 128
NO = N // P          # 16 partition tiles over the 2048 axis
MCW = 512            # m-chunk width (one PSUM bank)
MH = 1024            # m-half width for phase B
LN_EPS = 1e-5
NCORES = 8

# fp8 e3m4 pre-scales (values must sit in [2^-6, 15.5])
SC_V = 32.0          # Vs
SC_C = 32.0          # cheb
SC_B = 16.0          # bs
SC_X = 2.0           # x
SC_L = 16.0          # lhs2 (so sigmoid's input scale 1/16 also descales bs)
EXP_BIAS = -math.log(16.0)   # expS stored = exp(S_true)/16
RT_UNSCALE = 4.0     # rt out = rT_true/4  (psum 4*rT, evac scale 1/16)
CS_UNSCALE = 16.0    # cso = colsum/16

_BF16 = ml_dtypes.bfloat16
_E3M4 = ml_dtypes.float8_e3m4

CH_SH = K * N * N // NCORES   # cheb shard length (flat)
BS_SH = N * N // NCORES       # bs shard length (flat)


def _build_nc():
    import concourse.bass as bass
    import concourse.mybir as mybir
    import concourse.tile as tile

    nc = bass.Bass(num_devices=NCORES)
    bf16 = mybir.dt.bfloat16
    fp8 = mybir.dt.float8e3
    f32 = mybir.dt.float32
    groups8 = [list(range(NCORES))]

    vsh = nc.dram_tensor("vsh", [2, P, NO, P], fp8, kind="ExternalInput")
    chsh = nc.dram_tensor("chsh", [CH_SH], fp8, kind="ExternalInput")
    bssh = nc.dram_tensor("bssh", [BS_SH], fp8, kind="ExternalInput")
    xin = nc.dram_tensor("xin", [P, NO, FT], fp8, kind="ExternalInput")
    l2 = nc.dram_tensor("l2", [T, N], bf16, kind="ExternalInput")
    r2 = nc.dram_tensor("r2", [T, N], bf16, kind="ExternalInput")
    rt = nc.dram_tensor("rt", [K, FT, N], fp8, kind="ExternalOutput")
    cso = nc.dram_tensor("cso", [1, N], f32, kind="ExternalOutput")

    with tile.TileContext(nc) as tc, ExitStack() as ctx:
        dram = ctx.enter_context(tc.tile_pool(name="dram", bufs=1,
                                              space="DRAM"))
        singles = ctx.enter_context(tc.tile_pool(name="singles", bufs=1))
        vrpool = ctx.enter_context(tc.tile_pool(name="vrpool", bufs=2))
        vbpool = ctx.enter_context(tc.tile_pool(name="vbpool", bufs=2))
        bpool = ctx.enter_context(tc.tile_pool(name="bpool", bufs=3))
        cpool = ctx.enter_context(tc.tile_pool(name="cpool", bufs=3))
        apool = ctx.enter_context(tc.tile_pool(name="apool", bufs=3))
        evac = ctx.enter_context(tc.tile_pool(name="evac", bufs=8))
        psum = ctx.enter_context(tc.tile_pool(name="psum", bufs=8,
                                              space="PSUM"))

        # ---- DRAM bounces + AllGathers (ordered by first use: bs, Vs, cheb)
        bssh_b = dram.tile([BS_SH], fp8)
        vsh_b = dram.tile([2, P, NO, P], fp8)
        chsh_b = dram.tile([CH_SH], fp8)
        bs_g = dram.tile([N, N], fp8, addr_space="Shared")
        vst_g = dram.tile([NO, P, NO, P], fp8, addr_space="Shared")
        chb_g = dram.tile([K, N, N], fp8, addr_space="Shared")
        nc.gpsimd.dma_start(bssh_b[:], bssh[:])
        nc.gpsimd.dma_start(vsh_b[:, :, :, :], vsh[:, :, :, :])
        nc.gpsimd.dma_start(chsh_b[:], chsh[:])
        nc.gpsimd.collective_compute(
            "AllGather", mybir.AluOpType.bypass, replica_groups=groups8,
            ins=[bssh_b.opt()], outs=[bs_g.opt()])
        nc.gpsimd.collective_compute(
            "AllGather", mybir.AluOpType.bypass, replica_groups=groups8,
            ins=[vsh_b.opt()], outs=[vst_g.opt()])
        nc.gpsimd.collective_compute(
            "AllGather", mybir.AluOpType.bypass, replica_groups=groups8,
            ins=[chsh_b.opt()], outs=[chb_g.opt()])

        # ---- SBUF residents
        l2_sb = singles.tile([T, N], bf16)
        r2_sb = singles.tile([T, N], bf16)
        p_sb = singles.tile([P, NO, N], bf16)
        expS_sb = singles.tile([P, NO, N], fp8)
        colacc = singles.tile([P, N], f32)
        ones_sb = singles.tile([P, 1], f32)
        ebias = singles.tile([P, 1], f32)
        x_raw = singles.tile([P, NO, FT], fp8)
        x_sb = singles.tile([P, NO, FT], bf16)
        nc.sync.dma_start(l2_sb, l2[:, :])
        nc.sync.dma_start(r2_sb, r2[:, :])
        nc.sync.dma_start(x_raw, xin[:, :, :])
        nc.vector.memset(colacc, 0.0)
        nc.vector.memset(ones_sb, 1.0)
        nc.vector.memset(ebias, EXP_BIAS)

        # ---- P-phase: P = sigmoid((prod2*16 + bs*16) / 16) ----
        for io in range(NO):
            for q in range(4):
                ps = psum.tile([P, MCW], f32, tag="ps", name=f"pp{io}_{q}")
                nc.tensor.matmul(ps, l2_sb[:, io * P:(io + 1) * P],
                                 r2_sb[:, q * MCW:(q + 1) * MCW],
                                 start=True, stop=True)
                bs_t = bpool.tile([P, MCW], fp8, tag="bs")
                nc.sync.dma_start(
                    bs_t, bs_g[io * P:(io + 1) * P, q * MCW:(q + 1) * MCW])
                nc.vector.tensor_add(ps, ps, bs_t)
                nc.scalar.activation(
                    out=p_sb[:, io, q * MCW:(q + 1) * MCW], in_=ps,
                    func=mybir.ActivationFunctionType.Sigmoid,
                    scale=1.0 / SC_L)

        # x upconvert (ACT; before phase A evacs so phase B never waits)
        nc.scalar.add(x_sb, x_raw, 0.0)

        # ---- Phase A: S = (32Vs) @ P; expS = exp(S/32 - ln16) (fp8) ----
        for nb in range(NO):
            v_raw = vrpool.tile([P, NO, P], fp8, tag="vr")
            nc.sync.dma_start(v_raw, vst_g[nb, :, :, :])
            v_bf = vbpool.tile([P, NO, P], bf16, tag="vb")
            nc.scalar.add(v_bf, v_raw, 0.0)
            ps_q = [psum.tile([P, MCW], f32, tag="ps", name=f"s{nb}_{q}")
                    for q in range(4)]
            for io in range(NO):
                for q in range(4):
                    nc.tensor.matmul(
                        ps_q[q], v_bf[:, io, :],
                        p_sb[:, io, q * MCW:(q + 1) * MCW],
                        start=(io == 0), stop=(io == NO - 1))
            for q in range(4):
                nc.scalar.activation(
                    out=expS_sb[:, nb, q * MCW:(q + 1) * MCW], in_=ps_q[q],
                    func=mybir.ActivationFunctionType.Exp,
                    scale=1.0 / SC_V, bias=ebias)
            nc.vector.tensor_add(colacc, colacc, expS_sb[:, nb, :])

        # ---- Phase B: rT_k = (2x)^T @ ((32cheb_k)*expS); 6-bank groups ----
        groups = [(k, mh) for k in range(K) for mh in range(N // MH)]

        def _group_prologue(gi):
            k, mh = groups[gi]
            ms = mh * MH
            c_t = cpool.tile([P, MH], fp8, tag="c")
            nc.sync.dma_start(c_t, chb_g[k, 0:P, ms:ms + MH])
            a_t = apool.tile([P, MH], bf16, tag="a")
            nc.vector.tensor_mul(a_t, c_t, expS_sb[:, 0, ms:ms + MH])
            return a_t

        a_next = _group_prologue(0)
        for gi, (k, mh) in enumerate(groups):
            ms = mh * MH
            rt_ps = [[psum.tile([P, MCW], f32, tag="ps",
                                name=f"r{k}_{mh}_{f}_{c2}")
                      for c2 in range(2)] for f in range(3)]
            for nb in range(NO):
                if nb == 0:
                    a_t = a_next
                else:
                    c_t = cpool.tile([P, MH], fp8, tag="c")
                    nc.sync.dma_start(
                        c_t, chb_g[k, nb * P:(nb + 1) * P, ms:ms + MH])
                    a_t = apool.tile([P, MH], bf16, tag="a")
                    nc.vector.tensor_mul(a_t, c_t,
                                         expS_sb[:, nb, ms:ms + MH])
                for f in range(3):
                    for c2 in range(2):
                        nc.tensor.matmul(
                            rt_ps[f][c2],
                            x_sb[:, nb, f * P:(f + 1) * P],
                            a_t[:, c2 * MCW:(c2 + 1) * MCW],
                            start=(nb == 0), stop=(nb == NO - 1))
            if gi + 1 < len(groups):
                a_next = _group_prologue(gi + 1)
            for f in range(3):
                for c2 in range(2):
                    ev = evac.tile([P, MCW], fp8, tag="ev")
                    nc.scalar.activation(
                        out=ev, in_=rt_ps[f][c2],
                        func=mybir.ActivationFunctionType.Identity,
                        scale=1.0 / 16.0)
                    nc.scalar.dma_start(
                        rt[k, f * P:(f + 1) * P,
                           ms + c2 * MCW:ms + (c2 + 1) * MCW],
                        ev)

        # ---- Final column sums: ones^T @ colacc (fp32 matmuls) ----
        for q in range(4):
            cs_ps = psum.tile([1, MCW], f32, tag="ps", name=f"cs{q}")
            nc.tensor.matmul(cs_ps, ones_sb,
                             colacc[:, q * MCW:(q + 1) * MCW],
                             start=True, stop=True)
            cs_ev = evac.tile([1, MCW], f32, tag="csev")
            nc.vector.tensor_copy(out=cs_ev, in_=cs_ps)
            nc.scalar.dma_start(cso[:, q * MCW:(q + 1) * MCW], cs_ev)

    # TRN2 sequencers accept at most 1 sync wait per instruction (2 on
    # EventSemaphore); Tile emits multi-wait sync_info — this bacc
    # legalization pass splits the waits.
    import bass_rust
    bass_rust.generate_event_semaphores(nc)
    return nc


_NC_CACHE = None


def _get_nc():
    global _NC_CACHE
    if _NC_CACHE is None:
        _NC_CACHE = _build_nc()
    return _NC_CACHE


def _softmax(a, axis):
    m = a.max(axis=axis, keepdims=True)
    e = np.exp(a - m)
    return e / e.sum(axis=axis, keepdims=True)


def _host_factors(x, U1, U2, U3, be, Ve, W1, W2, W3):
    """Temporal attention + spatial-attention low-rank factors.

    Returns lhs2 [B,N,T], rhs2 [B,T,N] with P = sigmoid(lhs2@rhs2 + bs).
    """
    inner = np.einsum('bnft,n->btf', x, U1, optimize=True)        # [B,T,F]
    lhs = inner @ U2                                              # [B,T,N]
    rhs = np.einsum('f,bnft->bnt', U3, x, optimize=True)          # [B,N,T]
    prod = np.einsum('btn,bnu->btu', lhs, rhs, optimize=True)     # [B,T,T]
    E = np.matmul(Ve, 1.0 / (1.0 + np.exp(-(prod + be))))         # [B,T,T]
    tat = _softmax(E, axis=1)
    x_tat = (x.reshape(B, N * F, T) @ tat).reshape(B, N, F, T)
    lhs2 = np.einsum('bnft,t->bnf', x_tat, W1, optimize=True) @ W2
    rhs2 = np.einsum('f,bnft->btn', W3, x_tat, optimize=True)
    return lhs2, rhs2


def _prep_vst(Vs):
    """[NO, P, NO, P] e3m4: vst[nb, p, io, j] = 32*Vs[nb*128+j, io*128+p]."""
    return np.ascontiguousarray(
        (SC_V * Vs).reshape(NO, P, NO, P).transpose(0, 3, 2, 1)).astype(_E3M4)


def _prep_x(xb):
    """[P, NO, FT] e3m4: xin[p, nb, ft] = 2*xb[nb*128+p, ft]."""
    return np.ascontiguousarray(
        (SC_X * xb).reshape(NO, P, FT).transpose(1, 0, 2)).astype(_E3M4)


def _device_in_maps(x, lhs2, rhs2, Vs, cheb, bs):
    """Per-core input dicts (core b owns batch b + shard b of Vs/cheb/bs)."""
    vst = _prep_vst(Vs)                                   # [NO,P,NO,P] e3m4
    ch8 = (SC_C * cheb).astype(_E3M4).reshape(NCORES, CH_SH)
    bs8 = (SC_B * bs[0]).astype(_E3M4).reshape(NCORES, BS_SH)
    in_maps = []
    for b in range(B):
        in_maps.append({
            "vsh": np.ascontiguousarray(vst[2 * b:2 * b + 2]),
            "chsh": ch8[b],
            "bssh": bs8[b],
            "xin": _prep_x(x[b].reshape(N, FT)),
            "l2": np.ascontiguousarray(
                (SC_L * lhs2[b]).T).astype(_BF16),        # [T, N]
            "r2": np.ascontiguousarray(rhs2[b]).astype(_BF16),  # [T, N]
        })
    return in_maps


def _host_post(x, rT, cs, Theta, tconv_w, tconv_b, rconv_w, rconv_b,
               ln_gamma, ln_beta):
    """rT: [B, K, FT, N] f32 device output; cs: [B, N]; finish the block.

    Works in [*, T, N] layout so every contraction is a single GEMM.
    """
    Theta2 = np.ascontiguousarray(
        Theta.reshape(K * F, C).T)                    # [C, KF]
    Wt = tconv_w[:, :, 0, :]                          # [TF, C, 3]
    Wr = rconv_w[:, :, 0, 0]                          # [TF, F]
    y = np.empty((B, TF, T, N), np.float32)
    for b in range(B):
        # gcn[c, t, n] = relu(Theta^T @ r_norm)
        M = (rT[b] / cs[b]).reshape(K * F, T * N)
        gcn = np.maximum(Theta2 @ M, 0.0).reshape(C, T, N)
        gp = np.pad(gcn, ((0, 0), (1, 1), (0, 0)))    # pad t
        acc = Wt[:, :, 0] @ gp[:, 0:T, :].reshape(C, T * N)
        for dt in range(1, 3):
            acc += Wt[:, :, dt] @ np.ascontiguousarray(
                gp[:, dt:dt + T, :]).reshape(C, T * N)
        xb = np.ascontiguousarray(
            x[b].transpose(1, 2, 0)).reshape(F, T * N)  # [F, T*N]
        acc += Wr @ xb
        yb = acc.reshape(TF, T, N)
        yb += (tconv_b + rconv_b)[:, None, None]
        np.maximum(yb, 0.0, out=yb)
        mu = yb.mean(axis=0)
        var = yb.var(axis=0)
        yb -= mu
        yb *= 1.0 / np.sqrt(var + LN_EPS)
        yb *= ln_gamma[:, None, None]
        yb += ln_beta[:, None, None]
        y[b] = yb
    return np.ascontiguousarray(y.transpose(0, 3, 1, 2))  # [B, N, TF, T]


def _host_device_equiv(lhs2, rhs2, bs, Vs, cheb, x):
    """Pure-host fallback for the device stage (same math, f32)."""
    rT = np.zeros((B, K, FT, N), np.float32)
    cs = np.zeros((B, N), np.float32)
    for b in range(B):
        Pm = 1.0 / (1.0 + np.exp(-(lhs2[b] @ rhs2[b] + bs[0])))
        S = Vs @ Pm
        eS = np.exp(S)
        cs[b] = eS.sum(axis=0)
        xf = x[b].reshape(N, FT)
        for k in range(K):
            A = cheb[k] * eS
            rT[b, k] = xf.T @ A
    return rT, cs


def kernel(**inputs):
    x = np.asarray(inputs["x"], np.float32)
    cheb = np.asarray(inputs["cheb"], np.float32)
    U1 = np.asarray(inputs["U1"], np.float32)
    U2 = np.asarray(inputs["U2"], np.float32)
    U3 = np.asarray(inputs["U3"], np.float32)
    be = np.asarray(inputs["be"], np.float32)
    Ve = np.asarray(inputs["Ve"], np.float32)
    W1 = np.asarray(inputs["W1"], np.float32)
    W2 = np.asarray(inputs["W2"], np.float32)
    W3 = np.asarray(inputs["W3"], np.float32)
    bs = np.asarray(inputs["bs"], np.float32)
    Vs = np.asarray(inputs["Vs"], np.float32)
    Theta = np.asarray(inputs["Theta"], np.float32)
    tconv_w = np.asarray(inputs["tconv_w"], np.float32)
    tconv_b = np.asarray(inputs["tconv_b"], np.float32)
    rconv_w = np.asarray(inputs["rconv_w"], np.float32)
    rconv_b = np.asarray(inputs["rconv_b"], np.float32)
    ln_gamma = np.asarray(inputs["ln_gamma"], np.float32)
    ln_beta = np.asarray(inputs["ln_beta"], np.float32)

    lhs2, rhs2 = _host_factors(x, U1, U2, U3, be, Ve, W1, W2, W3)

    try:
        from concourse.bass_utils import run_bass_kernel_spmd
        nc = _get_nc()
        in_maps = _device_in_maps(x, lhs2, rhs2, Vs, cheb, bs)
        res = run_bass_kernel_spmd(nc, in_maps, core_ids=list(range(B)))
        rT = RT_UNSCALE * np.stack(
            [res.results[b]["rt"].astype(np.float32) for b in range(B)])
        cs = CS_UNSCALE * np.stack(
            [res.results[b]["cso"][0].astype(np.float32) for b in range(B)])
    except Exception as e:
        print(f"kernel.py: device path failed ({e!r}); host fallback",
              file=sys.stderr)
        rT, cs = _host_device_equiv(lhs2, rhs2, bs, Vs, cheb, x)

    return _host_post(x, rT, cs, Theta, tconv_w, tconv_b, rconv_w, rconv_b,
                      ln_gamma, ln_beta)


if __name__ == "__main__":
    import reference
    ins = {k: np.asarray(v) for k, v in reference.setup_inputs().items()}
    out = kernel(**ins)
    exp = np.asarray(reference.reference(**ins))
    err = np.abs(out - exp).max() / (np.abs(exp).max() + 1e-30)
    print("Relative error:", err)


# revision 6
# speedup vs baseline: 13.0781x; 4.9681x over previous
"""ASTGCN block kernel for 8 Trainium2 NeuronCores.

Strategy: data-parallel over batch B=8 (one batch element per core), with
all batch-invariant tensors (Vs, cheb, bs) shipped to the device SHARDED
(1/8 per core) and reconstructed on-device via DRAM AllGather over
NeuronLink — the host<->device link is the bottleneck for this problem, so
every unique byte crosses it exactly once.  The spatial-attention logits P
are NOT shipped at all: P = sigmoid(lhs2 @ rhs2 + bs) is rank-T (T=24), so
only the tiny factors (lhs2, rhs2) cross the link and the [N,N] sigmoid is
computed on device.  Large tensors travel as fp8 (e3m4) with power-of-two
pre-scales chosen so values sit in e3m4's [2^-6, 15.5] window; descales are
folded into ACT scale/bias operands (exp(x*s - ln16) = exp(x*s)/16), so
dequantization is free.  Measured end-to-end rel err ~5e-4 (tolerance 2e-2).

Device pipeline per core (batch b):
  P-phase:  prod2 = (16*lhs2_b)^T-contracted with rhs2_b (PE, contract=24)
            PSUM += 16*bs (DVE, fp8 operand); P = sigmoid(PSUM/16) (ACT)
  Phase A:  S = (32*Vs) @ P streamed from gathered DRAM (fp8 -> bf16 ACT
            upconvert, ldw amortized over 4 PSUM banks);
            expS = exp(S/32 - ln16) -> fp8 e3m4 (= exp(S_true)/16);
            colacc += expS (DVE f32)
  Phase B:  rT_k = (2x)^T @ ((32*cheb_k) * expS)  -- mask-mul on DVE with
            both operands fp8, output bf16 = 2*cheb*expS_true; PSUM = 4*rT;
            evac ACT scale 1/16 -> rt out = rT/4 in fp8 e3m4.
  colsum:   cso = ones^T @ colacc = colsum/16 (fp32 matmuls).
Host: temporal attention prologue (tiny [T,T] algebra) + lhs2/rhs2 factors
before; Theta contraction, temporal/residual convs, LayerNorm after.
"""

import sys
import math
import numpy as np
import ml_dtypes
from contextlib import ExitStack

B, N, F, T = 8, 2048, 16, 24
K, C, TF = 3, 64, 64
FT = F * T  # 384
P = 128
NO = N // P          # 16 partition tiles over the 2048 axis
MCW = 512            # m-chunk width (one PSUM bank)
MH = 1024            # m-half width for phase B
LN_EPS = 1e-5
NCORES = 8

# fp8 e3m4 pre-scales (values must sit in [2^-6, 15.5])
SC_V = 32.0          # Vs
SC_C = 32.0          # cheb
SC_B = 16.0          # bs
SC_X = 2.0           # x
SC_L = 16.0          # lhs2 (so sigmoid's input scale 1/16 also descales bs)
EXP_BIAS = -math.log(16.0)   # expS stored = exp(S_true)/16
RT_UNSCALE = 4.0     # rt out = rT_true/4  (psum 4*rT, evac scale 1/16)
CS_UNSCALE = 16.0    # cso = colsum/16

_BF16 = ml_dtypes.bfloat16
_E3M4 = ml_dtypes.float8_e3m4

CH_SH = K * N * N // NCORES   # cheb shard length (flat)
BS_SH = N * N // NCORES       # bs shard length (flat)
VS_SH = N * N // NCORES       # Vs shard length (flat, vst layout)

# Packed single-buffer I/O layout (byte offsets into the fp8 tensors).
# Fewer, larger buffers amortize the per-buffer launch overhead of the
# host<->device link.
OFF_VS = 0
OFF_CH = OFF_VS + VS_SH                   # 524288
OFF_BS = OFF_CH + CH_SH                   # 2097152
OFF_X = OFF_BS + BS_SH                    # 2621440
OFF_L2 = OFF_X + P * NO * FT              # 3407872
OFF_R2 = OFF_L2 + 2 * T * N               # 3506176
PKLEN = OFF_R2 + 2 * T * N                # 3604480
OFF_CS = K * FT * N                       # rt bytes, 2359296
POLEN = OFF_CS + 4 * N                    # + cso f32 bytes = 2367488


def _build_nc():
    import concourse.bass as bass
    import concourse.mybir as mybir
    import concourse.tile as tile

    nc = bass.Bass(num_devices=NCORES)
    bf16 = mybir.dt.bfloat16
    fp8 = mybir.dt.float8e3
    f32 = mybir.dt.float32
    groups8 = [list(range(NCORES))]

    pk = nc.dram_tensor("pk", [PKLEN], fp8, kind="ExternalInput")
    po = nc.dram_tensor("po", [POLEN], fp8, kind="ExternalOutput")
    rt = po[0:OFF_CS].rearrange("(k f n) -> k f n", f=FT, n=N)
    cso = po[OFF_CS:POLEN].bitcast(f32).rearrange("(a n) -> a n", n=N)
    xin = pk[OFF_X:OFF_L2].rearrange("(p a b) -> p a b", a=NO, b=FT)
    l2 = pk[OFF_L2:OFF_R2].bitcast(bf16).rearrange("(t n) -> t n", n=N)
    r2 = pk[OFF_R2:PKLEN].bitcast(bf16).rearrange("(t n) -> t n", n=N)

    with tile.TileContext(nc) as tc, ExitStack() as ctx:
        dram = ctx.enter_context(tc.tile_pool(name="dram", bufs=1,
                                              space="DRAM"))
        singles = ctx.enter_context(tc.tile_pool(name="singles", bufs=1))
        vrpool = ctx.enter_context(tc.tile_pool(name="vrpool", bufs=2))
        vbpool = ctx.enter_context(tc.tile_pool(name="vbpool", bufs=2))
        bpool = ctx.enter_context(tc.tile_pool(name="bpool", bufs=3))
        cpool = ctx.enter_context(tc.tile_pool(name="cpool", bufs=3))
        apool = ctx.enter_context(tc.tile_pool(name="apool", bufs=3))
        evac = ctx.enter_context(tc.tile_pool(name="evac", bufs=8))
        psum = ctx.enter_context(tc.tile_pool(name="psum", bufs=8,
                                              space="PSUM"))

        # ---- One DRAM bounce + AllGathers (ordered by first use: bs, Vs,
        # cheb).  Collectives can't touch I/O tensors, hence the bounce.
        sh_b = dram.tile([OFF_X], fp8)
        bs_g = dram.tile([N, N], fp8, addr_space="Shared")
        vst_g = dram.tile([NO, P, NO, P], fp8, addr_space="Shared")
        chb_g = dram.tile([K, N, N], fp8, addr_space="Shared")
        nc.gpsimd.dma_start(sh_b[:], pk[0:OFF_X])
        nc.gpsimd.collective_compute(
            "AllGather", mybir.AluOpType.bypass, replica_groups=groups8,
            ins=[sh_b[OFF_BS:OFF_X].opt()], outs=[bs_g.opt()])
        nc.gpsimd.collective_compute(
            "AllGather", mybir.AluOpType.bypass, replica_groups=groups8,
            ins=[sh_b[OFF_VS:OFF_CH].opt()], outs=[vst_g.opt()])
        nc.gpsimd.collective_compute(
            "AllGather", mybir.AluOpType.bypass, replica_groups=groups8,
            ins=[sh_b[OFF_CH:OFF_BS].opt()], outs=[chb_g.opt()])

        # ---- SBUF residents
        l2_sb = singles.tile([T, N], bf16)
        r2_sb = singles.tile([T, N], bf16)
        p_sb = singles.tile([P, NO, N], bf16)
        expS_sb = singles.tile([P, NO, N], fp8)
        colacc = singles.tile([P, N], f32)
        ones_sb = singles.tile([P, 1], f32)
        ebias = singles.tile([P, 1], f32)
        x_raw = singles.tile([P, NO, FT], fp8)
        x_sb = singles.tile([P, NO, FT], bf16)
        nc.sync.dma_start(l2_sb, l2[:, :])
        nc.sync.dma_start(r2_sb, r2[:, :])
        nc.sync.dma_start(x_raw, xin[:, :, :])
        nc.vector.memset(colacc, 0.0)
        nc.vector.memset(ones_sb, 1.0)
        nc.vector.memset(ebias, EXP_BIAS)

        # ---- P-phase: P = sigmoid((prod2*16 + bs*16) / 16) ----
        for io in range(NO):
            for q in range(4):
                ps = psum.tile([P, MCW], f32, tag="ps", name=f"pp{io}_{q}")
                nc.tensor.matmul(ps, l2_sb[:, io * P:(io + 1) * P],
                                 r2_sb[:, q * MCW:(q + 1) * MCW],
                                 start=True, stop=True)
                bs_t = bpool.tile([P, MCW], fp8, tag="bs")
                nc.sync.dma_start(
                    bs_t, bs_g[io * P:(io + 1) * P, q * MCW:(q + 1) * MCW])
                nc.vector.tensor_add(ps, ps, bs_t)
                nc.scalar.activation(
                    out=p_sb[:, io, q * MCW:(q + 1) * MCW], in_=ps,
                    func=mybir.ActivationFunctionType.Sigmoid,
                    scale=1.0 / SC_L)

        # x upconvert (ACT; before phase A evacs so phase B never waits)
        nc.scalar.add(x_sb, x_raw, 0.0)

        # ---- Phase A: S = (32Vs) @ P; expS = exp(S/32 - ln16) (fp8) ----
        for nb in range(NO):
            v_raw = vrpool.tile([P, NO, P], fp8, tag="vr")
            nc.sync.dma_start(v_raw, vst_g[nb, :, :, :])
            v_bf = vbpool.tile([P, NO, P], bf16, tag="vb")
            nc.scalar.add(v_bf, v_raw, 0.0)
            ps_q = [psum.tile([P, MCW], f32, tag="ps", name=f"s{nb}_{q}")
                    for q in range(4)]
            for io in range(NO):
                for q in range(4):
                    nc.tensor.matmul(
                        ps_q[q], v_bf[:, io, :],
                        p_sb[:, io, q * MCW:(q + 1) * MCW],
                        start=(io == 0), stop=(io == NO - 1))
            for q in range(4):
                nc.scalar.activation(
                    out=expS_sb[:, nb, q * MCW:(q + 1) * MCW], in_=ps_q[q],
                    func=mybir.ActivationFunctionType.Exp,
                    scale=1.0 / SC_V, bias=ebias)
            nc.vector.tensor_add(colacc, colacc, expS_sb[:, nb, :])

        # ---- Phase B: rT_k = (2x)^T @ ((32cheb_k)*expS); 6-bank groups ----
        groups = [(k, mh) for k in range(K) for mh in range(N // MH)]

        def _group_prologue(gi):
            k, mh = groups[gi]
            ms = mh * MH
            c_t = cpool.tile([P, MH], fp8, tag="c")
            nc.sync.dma_start(c_t, chb_g[k, 0:P, ms:ms + MH])
            a_t = apool.tile([P, MH], bf16, tag="a")
            nc.vector.tensor_mul(a_t, c_t, expS_sb[:, 0, ms:ms + MH])
            return a_t

        a_next = _group_prologue(0)
        for gi, (k, mh) in enumerate(groups):
            ms = mh * MH
            rt_ps = [[psum.tile([P, MCW], f32, tag="ps",
                                name=f"r{k}_{mh}_{f}_{c2}")
                      for c2 in range(2)] for f in range(3)]
            for nb in range(NO):
                if nb == 0:
                    a_t = a_next
                else:
                    c_t = cpool.tile([P, MH], fp8, tag="c")
                    nc.sync.dma_start(
                        c_t, chb_g[k, nb * P:(nb + 1) * P, ms:ms + MH])
                    a_t = apool.tile([P, MH], bf16, tag="a")
                    nc.vector.tensor_mul(a_t, c_t,
                                         expS_sb[:, nb, ms:ms + MH])
                for f in range(3):
                    for c2 in range(2):
                        nc.tensor.matmul(
                            rt_ps[f][c2],
                            x_sb[:, nb, f * P:(f + 1) * P],
                            a_t[:, c2 * MCW:(c2 + 1) * MCW],
                            start=(nb == 0), stop=(nb == NO - 1))
            if gi + 1 < len(groups):
                a_next = _group_prologue(gi + 1)
            for f in range(3):
                for c2 in range(2):
                    ev = evac.tile([P, MCW], fp8, tag="ev")
                    nc.scalar.activation(
                        out=ev, in_=rt_ps[f][c2],
                        func=mybir.ActivationFunctionType.Identity,
                        scale=1.0 / 16.0)
                    nc.scalar.dma_start(
                        rt[k, f * P:(f + 1) * P,
                           ms + c2 * MCW:ms + (c2 + 1) * MCW],
                        ev)

        # ---- Final column sums: ones^T @ colacc (fp32 matmuls) ----
        for q in range(4):
            cs_ps = psum.tile([1, MCW], f32, tag="ps", name=f"cs{q}")
            nc.tensor.matmul(cs_ps, ones_sb,
                             colacc[:, q * MCW:(q + 1) * MCW],
                             start=True, stop=True)
            cs_ev = evac.tile([1, MCW], f32, tag="csev")
            nc.vector.tensor_copy(out=cs_ev, in_=cs_ps)
            nc.scalar.dma_start(cso[:, q * MCW:(q + 1) * MCW], cs_ev)

    # TRN2 sequencers accept at most 1 sync wait per instruction (2 on
    # EventSemaphore); Tile emits multi-wait sync_info — this bacc
    # legalization pass splits the waits.
    import bass_rust
    bass_rust.generate_event_semaphores(nc)
    return nc


_NC_CACHE = None


def _get_nc():
    global _NC_CACHE
    if _NC_CACHE is None:
        _NC_CACHE = _build_nc()
    return _NC_CACHE


def _softmax(a, axis):
    m = a.max(axis=axis, keepdims=True)
    e = np.exp(a - m)
    return e / e.sum(axis=axis, keepdims=True)


def _host_factors(x, U1, U2, U3, be, Ve, W1, W2, W3):
    """Temporal attention + spatial-attention low-rank factors.

    Returns lhs2 [B,N,T], rhs2 [B,T,N] with P = sigmoid(lhs2@rhs2 + bs).
    """
    inner = np.einsum('bnft,n->btf', x, U1, optimize=True)        # [B,T,F]
    lhs = inner @ U2                                              # [B,T,N]
    rhs = np.einsum('f,bnft->bnt', U3, x, optimize=True)          # [B,N,T]
    prod = np.einsum('btn,bnu->btu', lhs, rhs, optimize=True)     # [B,T,T]
    E = np.matmul(Ve, 1.0 / (1.0 + np.exp(-(prod + be))))         # [B,T,T]
    tat = _softmax(E, axis=1)
    x_tat = (x.reshape(B, N * F, T) @ tat).reshape(B, N, F, T)
    lhs2 = np.einsum('bnft,t->bnf', x_tat, W1, optimize=True) @ W2
    rhs2 = np.einsum('f,bnft->btn', W3, x_tat, optimize=True)
    return lhs2, rhs2


def _prep_vst(Vs):
    """[NO, P, NO, P] e3m4: vst[nb, p, io, j] = 32*Vs[nb*128+j, io*128+p]."""
    return np.ascontiguousarray(
        (SC_V * Vs).reshape(NO, P, NO, P).transpose(0, 3, 2, 1)).astype(_E3M4)


def _prep_x(xb):
    """[P, NO, FT] e3m4: xin[p, nb, ft] = 2*xb[nb*128+p, ft]."""
    return np.ascontiguousarray(
        (SC_X * xb).reshape(NO, P, FT).transpose(1, 0, 2)).astype(_E3M4)


def _device_in_maps(x, lhs2, rhs2, Vs, cheb, bs):
    """Per-core packed input (core b owns batch b + shard b of Vs/cheb/bs)."""
    vst = _prep_vst(Vs).reshape(NCORES, VS_SH)            # [8, VS_SH] e3m4
    ch8 = (SC_C * cheb).astype(_E3M4).reshape(NCORES, CH_SH)
    bs8 = (SC_B * bs[0]).astype(_E3M4).reshape(NCORES, BS_SH)
    in_maps = []
    for b in range(B):
        parts = [
            vst[b].view(np.uint8),
            ch8[b].view(np.uint8),
            bs8[b].view(np.uint8),
            _prep_x(x[b].reshape(N, FT)).reshape(-1).view(np.uint8),
            np.ascontiguousarray((SC_L * lhs2[b]).T).astype(
                _BF16).reshape(-1).view(np.uint8),
            np.ascontiguousarray(rhs2[b]).astype(
                _BF16).reshape(-1).view(np.uint8),
        ]
        in_maps.append({"pk": np.concatenate(parts).view(_E3M4)})
    return in_maps


def _unpack_out(po):
    """po: packed fp8 [POLEN] -> (rT [K,FT,N] f32, cs [N] f32)."""
    po = np.ascontiguousarray(po)
    rT = RT_UNSCALE * po[:OFF_CS].astype(np.float32).reshape(K, FT, N)
    cs = CS_UNSCALE * po.view(np.uint8)[OFF_CS:].view(np.float32).copy()
    return rT, cs


def _host_post(x, rT, cs, Theta, tconv_w, tconv_b, rconv_w, rconv_b,
               ln_gamma, ln_beta):
    """rT: [B, K, FT, N] f32 device output; cs: [B, N]; finish the block.

    Works in [*, T, N] layout so every contraction is a single GEMM.
    """
    Theta2 = np.ascontiguousarray(
        Theta.reshape(K * F, C).T)                    # [C, KF]
    Wt = tconv_w[:, :, 0, :]                          # [TF, C, 3]
    Wr = rconv_w[:, :, 0, 0]                          # [TF, F]
    y = np.empty((B, TF, T, N), np.float32)
    for b in range(B):
        # gcn[c, t, n] = relu(Theta^T @ r_norm)
        M = (rT[b] / cs[b]).reshape(K * F, T * N)
        gcn = np.maximum(Theta2 @ M, 0.0).reshape(C, T, N)
        gp = np.pad(gcn, ((0, 0), (1, 1), (0, 0)))    # pad t
        acc = Wt[:, :, 0] @ gp[:, 0:T, :].reshape(C, T * N)
        for dt in range(1, 3):
            acc += Wt[:, :, dt] @ np.ascontiguousarray(
                gp[:, dt:dt + T, :]).reshape(C, T * N)
        xb = np.ascontiguousarray(
            x[b].transpose(1, 2, 0)).reshape(F, T * N)  # [F, T*N]
        acc += Wr @ xb
        yb = acc.reshape(TF, T, N)
        yb += (tconv_b + rconv_b)[:, None, None]
        np.maximum(yb, 0.0, out=yb)
        mu = yb.mean(axis=0)
        var = yb.var(axis=0)
        yb -= mu
        yb *= 1.0 / np.sqrt(var + LN_EPS)
        yb *= ln_gamma[:, None, None]
        yb += ln_beta[:, None, None]
        y[b] = yb
    return np.ascontiguousarray(y.transpose(0, 3, 1, 2))  # [B, N, TF, T]


def _host_device_equiv(lhs2, rhs2, bs, Vs, cheb, x):
    """Pure-host fallback for the device stage (same math, f32)."""
    rT = np.zeros((B, K, FT, N), np.float32)
    cs = np.zeros((B, N), np.float32)
    for b in range(B):
        Pm = 1.0 / (1.0 + np.exp(-(lhs2[b] @ rhs2[b] + bs[0])))
        S = Vs @ Pm
        eS = np.exp(S)
        cs[b] = eS.sum(axis=0)
        xf = x[b].reshape(N, FT)
        for k in range(K):
            A = cheb[k] * eS
            rT[b, k] = xf.T @ A
    return rT, cs


def kernel(**inputs):
    x = np.asarray(inputs["x"], np.float32)
    cheb = np.asarray(inputs["cheb"], np.float32)
    U1 = np.asarray(inputs["U1"], np.float32)
    U2 = np.asarray(inputs["U2"], np.float32)
    U3 = np.asarray(inputs["U3"], np.float32)
    be = np.asarray(inputs["be"], np.float32)
    Ve = np.asarray(inputs["Ve"], np.float32)
    W1 = np.asarray(inputs["W1"], np.float32)
    W2 = np.asarray(inputs["W2"], np.float32)
    W3 = np.asarray(inputs["W3"], np.float32)
    bs = np.asarray(inputs["bs"], np.float32)
    Vs = np.asarray(inputs["Vs"], np.float32)
    Theta = np.asarray(inputs["Theta"], np.float32)
    tconv_w = np.asarray(inputs["tconv_w"], np.float32)
    tconv_b = np.asarray(inputs["tconv_b"], np.float32)
    rconv_w = np.asarray(inputs["rconv_w"], np.float32)
    rconv_b = np.asarray(inputs["rconv_b"], np.float32)
    ln_gamma = np.asarray(inputs["ln_gamma"], np.float32)
    ln_beta = np.asarray(inputs["ln_beta"], np.float32)

    lhs2, rhs2 = _host_factors(x, U1, U2, U3, be, Ve, W1, W2, W3)

    try:
        from concourse.bass_utils import run_bass_kernel_spmd
        nc = _get_nc()
        in_maps = _device_in_maps(x, lhs2, rhs2, Vs, cheb, bs)
        res = run_bass_kernel_spmd(nc, in_maps, core_ids=list(range(B)))
        pairs = [_unpack_out(res.results[b]["po"]) for b in range(B)]
        rT = np.stack([p[0] for p in pairs])
        cs = np.stack([p[1] for p in pairs])
    except Exception as e:
        print(f"kernel.py: device path failed ({e!r}); host fallback",
              file=sys.stderr)
        rT, cs = _host_device_equiv(lhs2, rhs2, bs, Vs, cheb, x)

    return _host_post(x, rT, cs, Theta, tconv_w, tconv_b, rconv_w, rconv_b,
                      ln_gamma, ln_beta)


if __name__ == "__main__":
    import reference
    ins = {k: np.asarray(v) for k, v in reference.setup_inputs().items()}
    out = kernel(**ins)
    exp = np.asarray(reference.reference(**ins))
    err = np.abs(out - exp).max() / (np.abs(exp).max() + 1e-30)
    print("Relative error:", err)


# revision 16
# speedup vs baseline: 20.7267x; 1.5848x over previous
"""ASTGCN block kernel for 8 Trainium2 NeuronCores.

Strategy: data-parallel over batch B=8 (one batch element per core), with
all batch-invariant tensors (Vs, cheb, bs) shipped to the device SHARDED
(1/8 per core) and reconstructed on-device via DRAM AllGather over
NeuronLink — the host<->device link is the bottleneck for this problem, so
every unique byte crosses it exactly once.  The spatial-attention logits P
are NOT shipped at all: P = sigmoid(lhs2 @ rhs2 + bs) is rank-T (T=24), so
only the tiny factors (lhs2, rhs2) cross the link and the [N,N] sigmoid is
computed on device.  Large tensors travel as fp8 (e3m4) with power-of-two
pre-scales chosen so values sit in e3m4's [2^-6, 15.5] window; descales are
folded into ACT scale/bias operands (exp(x*s - ln16) = exp(x*s)/16), so
dequantization is free.  Measured end-to-end rel err ~5e-4 (tolerance 2e-2).

Device pipeline per core (batch b):
  P-phase:  prod2 = (16*lhs2_b)^T-contracted with rhs2_b (PE, contract=24)
            PSUM += 16*bs (DVE, fp8 operand); P = sigmoid(PSUM/16) (ACT)
  Phase A:  S = (32*Vs) @ P streamed from gathered DRAM (fp8 -> bf16 ACT
            upconvert, ldw amortized over 4 PSUM banks);
            expS = exp(S/32 - ln16) -> fp8 e3m4 (= exp(S_true)/16);
            colacc += expS (DVE f32)
  Phase B:  rT_k = (2x)^T @ ((32*cheb_k) * expS)  -- mask-mul on DVE with
            both operands fp8, output bf16 = 2*cheb*expS_true; PSUM = 4*rT;
            evac ACT scale 1/16 -> rt out = rT/4 in fp8 e3m4.
  colsum:   cso = ones^T @ colacc = colsum/16 (fp32 matmuls).
Host: temporal attention prologue (tiny [T,T] algebra) + lhs2/rhs2 factors
before; Theta contraction, temporal/residual convs, LayerNorm after.
"""

import sys
import math
import numpy as np
import ml_dtypes
from contextlib import ExitStack

B, N, F, T = 8, 2048, 16, 24
K, C, TF = 3, 64, 64
FT = F * T  # 384
P = 128
NO = N // P          # 16 partition tiles over the 2048 axis
MCW = 512            # m-chunk width (one PSUM bank)
MH = 1024            # m-half width for phase B
LN_EPS = 1e-5
NCORES = 8

# fp8 e3m4 pre-scales (values must sit in [2^-6, 15.5])
SC_V = 32.0          # Vs
SC_C = 32.0          # cheb
SC_B = 16.0          # bs
SC_X = 2.0           # x
SC_L = 16.0          # lhs2 (so sigmoid's input scale 1/16 also descales bs)
EXP_BIAS = -math.log(16.0)   # expS stored = exp(S_true)/16
RT_UNSCALE = 4.0     # rt out = rT_true/4  (psum 4*rT, evac scale 1/16)
CS_UNSCALE = 16.0    # cso = colsum/16

_BF16 = ml_dtypes.bfloat16
_E3M4 = ml_dtypes.float8_e3m4

CH_SH = K * N * N // NCORES   # cheb shard length (flat)
BS_SH = N * N // NCORES       # bs shard length (flat)
VS_SH = N * N // NCORES       # Vs shard length (flat, vst layout)

# Packed single-buffer I/O layout (byte offsets into the fp8 tensors).
# Fewer, larger buffers amortize the per-buffer launch overhead of the
# host<->device link.
OFF_VS = 0
OFF_CH = OFF_VS + VS_SH                   # 524288
OFF_BS = OFF_CH + CH_SH                   # 2097152
OFF_X = OFF_BS + BS_SH                    # 2621440
OFF_L2 = OFF_X + P * NO * FT              # 3407872
OFF_R2 = OFF_L2 + 2 * T * N               # 3506176
PKLEN = OFF_R2 + 2 * T * N                # 3604480
OFF_CS = K * FT * N                       # rt bytes, 2359296
POLEN = OFF_CS + 4 * N                    # + cso f32 bytes = 2367488


def _build_nc():
    import concourse.bass as bass
    import concourse.mybir as mybir
    import concourse.tile as tile

    nc = bass.Bass(num_devices=NCORES)
    bf16 = mybir.dt.bfloat16
    fp8 = mybir.dt.float8e3
    f32 = mybir.dt.float32
    groups8 = [list(range(NCORES))]

    pk = nc.dram_tensor("pk", [PKLEN], fp8, kind="ExternalInput")
    po = nc.dram_tensor("po", [POLEN], fp8, kind="ExternalOutput")
    rt = po[0:OFF_CS].rearrange("(k f n) -> k f n", f=FT, n=N)
    cso = po[OFF_CS:POLEN].bitcast(f32).rearrange("(a n) -> a n", n=N)
    xin = pk[OFF_X:OFF_L2].rearrange("(p a b) -> p a b", a=NO, b=FT)
    l2 = pk[OFF_L2:OFF_R2].bitcast(bf16).rearrange("(t n) -> t n", n=N)
    r2 = pk[OFF_R2:PKLEN].bitcast(bf16).rearrange("(t n) -> t n", n=N)

    with tile.TileContext(nc) as tc, ExitStack() as ctx:
        dram = ctx.enter_context(tc.tile_pool(name="dram", bufs=1,
                                              space="DRAM"))
        singles = ctx.enter_context(tc.tile_pool(name="singles", bufs=1))
        vrpool = ctx.enter_context(tc.tile_pool(name="vrpool", bufs=2))
        vbpool = ctx.enter_context(tc.tile_pool(name="vbpool", bufs=2))
        bpool = ctx.enter_context(tc.tile_pool(name="bpool", bufs=3))
        cpool = ctx.enter_context(tc.tile_pool(name="cpool", bufs=3))
        apool = ctx.enter_context(tc.tile_pool(name="apool", bufs=3))
        evac = ctx.enter_context(tc.tile_pool(name="evac", bufs=8))
        psum = ctx.enter_context(tc.tile_pool(name="psum", bufs=8,
                                              space="PSUM"))

        # ---- One DRAM bounce + AllGathers (ordered by first use: bs, Vs,
        # cheb).  Collectives can't touch I/O tensors, hence the bounce.
        sh_b = dram.tile([OFF_X], fp8)
        bs_g = dram.tile([N, N], fp8, addr_space="Shared")
        vst_g = dram.tile([NO, P, NO, P], fp8, addr_space="Shared")
        chb_g = dram.tile([K, N, N], fp8, addr_space="Shared")
        nc.gpsimd.dma_start(sh_b[:], pk[0:OFF_X])
        nc.gpsimd.collective_compute(
            "AllGather", mybir.AluOpType.bypass, replica_groups=groups8,
            ins=[sh_b[OFF_BS:OFF_X].opt()], outs=[bs_g.opt()])
        nc.gpsimd.collective_compute(
            "AllGather", mybir.AluOpType.bypass, replica_groups=groups8,
            ins=[sh_b[OFF_VS:OFF_CH].opt()], outs=[vst_g.opt()])
        nc.gpsimd.collective_compute(
            "AllGather", mybir.AluOpType.bypass, replica_groups=groups8,
            ins=[sh_b[OFF_CH:OFF_BS].opt()], outs=[chb_g.opt()])

        # ---- SBUF residents
        l2_sb = singles.tile([T, N], bf16)
        r2_sb = singles.tile([T, N], bf16)
        p_sb = singles.tile([P, NO, N], bf16)
        expS_sb = singles.tile([P, NO, N], fp8)
        colacc = singles.tile([P, N], f32)
        ones_sb = singles.tile([P, 1], f32)
        ebias = singles.tile([P, 1], f32)
        x_raw = singles.tile([P, NO, FT], fp8)
        x_sb = singles.tile([P, NO, FT], bf16)
        nc.sync.dma_start(l2_sb, l2[:, :])
        nc.sync.dma_start(r2_sb, r2[:, :])
        nc.sync.dma_start(x_raw, xin[:, :, :])
        nc.vector.memset(colacc, 0.0)
        nc.vector.memset(ones_sb, 1.0)
        nc.vector.memset(ebias, EXP_BIAS)

        # ---- P-phase: P = sigmoid((prod2*16 + bs*16) / 16) ----
        for io in range(NO):
            for q in range(4):
                ps = psum.tile([P, MCW], f32, tag="ps", name=f"pp{io}_{q}")
                nc.tensor.matmul(ps, l2_sb[:, io * P:(io + 1) * P],
                                 r2_sb[:, q * MCW:(q + 1) * MCW],
                                 start=True, stop=True)
                bs_t = bpool.tile([P, MCW], fp8, tag="bs")
                nc.sync.dma_start(
                    bs_t, bs_g[io * P:(io + 1) * P, q * MCW:(q + 1) * MCW])
                nc.vector.tensor_add(ps, ps, bs_t)
                nc.scalar.activation(
                    out=p_sb[:, io, q * MCW:(q + 1) * MCW], in_=ps,
                    func=mybir.ActivationFunctionType.Sigmoid,
                    scale=1.0 / SC_L)

        # x upconvert (ACT; before phase A evacs so phase B never waits)
        nc.scalar.add(x_sb, x_raw, 0.0)

        # ---- Phase A: S = (32Vs) @ P; expS = exp(S/32 - ln16) (fp8) ----
        for nb in range(NO):
            v_raw = vrpool.tile([P, NO, P], fp8, tag="vr")
            nc.sync.dma_start(v_raw, vst_g[nb, :, :, :])
            v_bf = vbpool.tile([P, NO, P], bf16, tag="vb")
            nc.scalar.add(v_bf, v_raw, 0.0)
            ps_q = [psum.tile([P, MCW], f32, tag="ps", name=f"s{nb}_{q}")
                    for q in range(4)]
            for io in range(NO):
                for q in range(4):
                    nc.tensor.matmul(
                        ps_q[q], v_bf[:, io, :],
                        p_sb[:, io, q * MCW:(q + 1) * MCW],
                        start=(io == 0), stop=(io == NO - 1))
            for q in range(4):
                nc.scalar.activation(
                    out=expS_sb[:, nb, q * MCW:(q + 1) * MCW], in_=ps_q[q],
                    func=mybir.ActivationFunctionType.Exp,
                    scale=1.0 / SC_V, bias=ebias)
            nc.vector.tensor_add(colacc, colacc, expS_sb[:, nb, :])

        # ---- Phase B: rT_k = (2x)^T @ ((32cheb_k)*expS); 6-bank groups ----
        groups = [(k, mh) for k in range(K) for mh in range(N // MH)]

        def _group_prologue(gi):
            k, mh = groups[gi]
            ms = mh * MH
            c_t = cpool.tile([P, MH], fp8, tag="c")
            nc.sync.dma_start(c_t, chb_g[k, 0:P, ms:ms + MH])
            a_t = apool.tile([P, MH], bf16, tag="a")
            nc.vector.tensor_mul(a_t, c_t, expS_sb[:, 0, ms:ms + MH])
            return a_t

        a_next = _group_prologue(0)
        for gi, (k, mh) in enumerate(groups):
            ms = mh * MH
            rt_ps = [[psum.tile([P, MCW], f32, tag="ps",
                                name=f"r{k}_{mh}_{f}_{c2}")
                      for c2 in range(2)] for f in range(3)]
            for nb in range(NO):
                if nb == 0:
                    a_t = a_next
                else:
                    c_t = cpool.tile([P, MH], fp8, tag="c")
                    nc.sync.dma_start(
                        c_t, chb_g[k, nb * P:(nb + 1) * P, ms:ms + MH])
                    a_t = apool.tile([P, MH], bf16, tag="a")
                    nc.vector.tensor_mul(a_t, c_t,
                                         expS_sb[:, nb, ms:ms + MH])
                for f in range(3):
                    for c2 in range(2):
                        nc.tensor.matmul(
                            rt_ps[f][c2],
                            x_sb[:, nb, f * P:(f + 1) * P],
                            a_t[:, c2 * MCW:(c2 + 1) * MCW],
                            start=(nb == 0), stop=(nb == NO - 1))
            if gi + 1 < len(groups):
                a_next = _group_prologue(gi + 1)
            for f in range(3):
                for c2 in range(2):
                    ev = evac.tile([P, MCW], fp8, tag="ev")
                    nc.scalar.activation(
                        out=ev, in_=rt_ps[f][c2],
                        func=mybir.ActivationFunctionType.Identity,
                        scale=1.0 / 16.0)
                    nc.scalar.dma_start(
                        rt[k, f * P:(f + 1) * P,
                           ms + c2 * MCW:ms + (c2 + 1) * MCW],
                        ev)

        # ---- Final column sums: ones^T @ colacc (fp32 matmuls) ----
        for q in range(4):
            cs_ps = psum.tile([1, MCW], f32, tag="ps", name=f"cs{q}")
            nc.tensor.matmul(cs_ps, ones_sb,
                             colacc[:, q * MCW:(q + 1) * MCW],
                             start=True, stop=True)
            cs_ev = evac.tile([1, MCW], f32, tag="csev")
            nc.vector.tensor_copy(out=cs_ev, in_=cs_ps)
            nc.scalar.dma_start(cso[:, q * MCW:(q + 1) * MCW], cs_ev)

    # TRN2 sequencers accept at most 1 sync wait per instruction (2 on
    # EventSemaphore); Tile emits multi-wait sync_info — this bacc
    # legalization pass splits the waits.
    import bass_rust
    bass_rust.generate_event_semaphores(nc)
    return nc


_NC_CACHE = None


def _get_nc():
    global _NC_CACHE
    if _NC_CACHE is None:
        _NC_CACHE = _build_nc()
    return _NC_CACHE


def _softmax(a, axis):
    m = a.max(axis=axis, keepdims=True)
    e = np.exp(a - m)
    return e / e.sum(axis=axis, keepdims=True)


def _host_factors(x, U1, U2, U3, be, Ve, W1, W2, W3):
    """Temporal attention + spatial-attention low-rank factors.

    Returns lhs2 [B,N,T], rhs2 [B,T,N] with P = sigmoid(lhs2@rhs2 + bs).
    """
    inner = np.einsum('bnft,n->btf', x, U1, optimize=True)        # [B,T,F]
    lhs = inner @ U2                                              # [B,T,N]
    rhs = np.einsum('f,bnft->bnt', U3, x, optimize=True)          # [B,N,T]
    prod = np.einsum('btn,bnu->btu', lhs, rhs, optimize=True)     # [B,T,T]
    E = np.matmul(Ve, 1.0 / (1.0 + np.exp(-(prod + be))))         # [B,T,T]
    tat = _softmax(E, axis=1)
    x_tat = (x.reshape(B, N * F, T) @ tat).reshape(B, N, F, T)
    lhs2 = np.einsum('bnft,t->bnf', x_tat, W1, optimize=True) @ W2
    rhs2 = np.einsum('f,bnft->btn', W3, x_tat, optimize=True)
    return lhs2, rhs2


def _prep_vst(Vs):
    """[NO, P, NO, P] e3m4: vst[nb, p, io, j] = 32*Vs[nb*128+j, io*128+p]."""
    return np.ascontiguousarray(
        (SC_V * Vs).reshape(NO, P, NO, P).transpose(0, 3, 2, 1)).astype(_E3M4)


def _prep_x(xb):
    """[P, NO, FT] e3m4: xin[p, nb, ft] = 2*xb[nb*128+p, ft]."""
    return np.ascontiguousarray(
        (SC_X * xb).reshape(NO, P, FT).transpose(1, 0, 2)).astype(_E3M4)


def _device_in_maps(x, lhs2, rhs2, Vs, cheb, bs):
    """Per-core packed input (core b owns batch b + shard b of Vs/cheb/bs)."""
    vst = _prep_vst(Vs).reshape(NCORES, VS_SH)            # [8, VS_SH] e3m4
    ch8 = (SC_C * cheb).astype(_E3M4).reshape(NCORES, CH_SH)
    bs8 = (SC_B * bs[0]).astype(_E3M4).reshape(NCORES, BS_SH)
    in_maps = []
    for b in range(B):
        parts = [
            vst[b].view(np.uint8),
            ch8[b].view(np.uint8),
            bs8[b].view(np.uint8),
            _prep_x(x[b].reshape(N, FT)).reshape(-1).view(np.uint8),
            np.ascontiguousarray((SC_L * lhs2[b]).T).astype(
                _BF16).reshape(-1).view(np.uint8),
            np.ascontiguousarray(rhs2[b]).astype(
                _BF16).reshape(-1).view(np.uint8),
        ]
        in_maps.append({"pk": np.concatenate(parts).view(_E3M4)})
    return in_maps


def _unpack_out(po):
    """po: packed fp8 [POLEN] -> (rT [K,FT,N] f32, cs [N] f32)."""
    po = np.ascontiguousarray(po)
    rT = RT_UNSCALE * po[:OFF_CS].astype(np.float32).reshape(K, FT, N)
    cs = CS_UNSCALE * po.view(np.uint8)[OFF_CS:].view(np.float32).copy()
    return rT, cs


def _host_post(x, rT, cs, Theta, tconv_w, tconv_b, rconv_w, rconv_b,
               ln_gamma, ln_beta):
    """rT: [B, K, FT, N] f32 device output; cs: [B, N]; finish the block.

    Works in [*, T, N] layout so every contraction is a single GEMM.
    """
    Theta2 = np.ascontiguousarray(
        Theta.reshape(K * F, C).T)                    # [C, KF]
    Wt = tconv_w[:, :, 0, :]                          # [TF, C, 3]
    Wr = rconv_w[:, :, 0, 0]                          # [TF, F]
    y = np.empty((B, TF, T, N), np.float32)
    for b in range(B):
        # gcn[c, t, n] = relu(Theta^T @ r_norm)
        M = (rT[b] / cs[b]).reshape(K * F, T * N)
        gcn = np.maximum(Theta2 @ M, 0.0).reshape(C, T, N)
        gp = np.pad(gcn, ((0, 0), (1, 1), (0, 0)))    # pad t
        acc = Wt[:, :, 0] @ gp[:, 0:T, :].reshape(C, T * N)
        for dt in range(1, 3):
            acc += Wt[:, :, dt] @ np.ascontiguousarray(
                gp[:, dt:dt + T, :]).reshape(C, T * N)
        xb = np.ascontiguousarray(
            x[b].transpose(1, 2, 0)).reshape(F, T * N)  # [F, T*N]
        acc += Wr @ xb
        yb = acc.reshape(TF, T, N)
        yb += (tconv_b + rconv_b)[:, None, None]
        np.maximum(yb, 0.0, out=yb)
        mu = yb.mean(axis=0)
        var = yb.var(axis=0)
        yb -= mu
        yb *= 1.0 / np.sqrt(var + LN_EPS)
        yb *= ln_gamma[:, None, None]
        yb += ln_beta[:, None, None]
        y[b] = yb
    return np.ascontiguousarray(y.transpose(0, 3, 1, 2))  # [B, N, TF, T]


def _host_device_equiv(lhs2, rhs2, bs, Vs, cheb, x):
    """Pure-host fallback for the device stage (same math, f32)."""
    rT = np.zeros((B, K, FT, N), np.float32)
    cs = np.zeros((B, N), np.float32)
    for b in range(B):
        Pm = 1.0 / (1.0 + np.exp(-(lhs2[b] @ rhs2[b] + bs[0])))
        S = Vs @ Pm
        eS = np.exp(S)
        cs[b] = eS.sum(axis=0)
        xf = x[b].reshape(N, FT)
        for k in range(K):
            A = cheb[k] * eS
            rT[b, k] = xf.T @ A
    return rT, cs


def kernel(**inputs):
    x = np.asarray(inputs["x"], np.float32)
    cheb = np.asarray(inputs["cheb"], np.float32)
    U1 = np.asarray(inputs["U1"], np.float32)
    U2 = np.asarray(inputs["U2"], np.float32)
    U3 = np.asarray(inputs["U3"], np.float32)
    be = np.asarray(inputs["be"], np.float32)
    Ve = np.asarray(inputs["Ve"], np.float32)
    W1 = np.asarray(inputs["W1"], np.float32)
    W2 = np.asarray(inputs["W2"], np.float32)
    W3 = np.asarray(inputs["W3"], np.float32)
    bs = np.asarray(inputs["bs"], np.float32)
    Vs = np.asarray(inputs["Vs"], np.float32)
    Theta = np.asarray(inputs["Theta"], np.float32)
    tconv_w = np.asarray(inputs["tconv_w"], np.float32)
    tconv_b = np.asarray(inputs["tconv_b"], np.float32)
    rconv_w = np.asarray(inputs["rconv_w"], np.float32)
    rconv_b = np.asarray(inputs["rconv_b"], np.float32)
    ln_gamma = np.asarray(inputs["ln_gamma"], np.float32)
    ln_beta = np.asarray(inputs["ln_beta"], np.float32)

    lhs2, rhs2 = _host_factors(x, U1, U2, U3, be, Ve, W1, W2, W3)

    try:
        from concourse.bass_utils import run_bass_kernel_spmd
        nc = _get_nc()
        in_maps = _device_in_maps(x, lhs2, rhs2, Vs, cheb, bs)
        res = run_bass_kernel_spmd(nc, in_maps, core_ids=list(range(B)))
        pairs = [_unpack_out(res.results[b]["po"]) for b in range(B)]
        rT = np.stack([p[0] for p in pairs])
        cs = np.stack([p[1] for p in pairs])
    except Exception as e:
        print(f"kernel.py: device path failed ({e!r}); host fallback",
              file=sys.stderr)
        rT, cs = _host_device_equiv(lhs2, rhs2, bs, Vs, cheb, x)

    return _host_post(x, rT, cs, Theta, tconv_w, tconv_b, rconv_w, rconv_b,
                      ln_gamma, ln_beta)


if __name__ == "__main__":
    import reference
    ins = {k: np.asarray(v) for k, v in reference.setup_inputs().items()}
    out = kernel(**ins)
    exp = np.asarray(reference.reference(**ins))
    err = np.abs(out - exp).max() / (np.abs(exp).max() + 1e-30)
    print("Relative error:", err)
